# revision 22
# baseline (speedup 1.0000x reference)
"""Trainium2 Bass kernel for the BiLSTM-CRF loss (sum reduction).

Strategy (v4):
- Data-parallel: batch 256 sharded as 32 per NeuronCore across 8 cores.
- Host passes emissions pre-transposed to [T, (step j, seg k, batch b)] and
  pre-cast to bf16 (layout/dtype prep only; all math stays on device), plus
  a f32 copy for the exact numerator gather. This removes all on-device PE
  transposes and DVE casts and halves streamed HBM traffic.
- Normalizer (forward algorithm) runs in LINEAR space: alpha_{s+1} =
  exp(em_{s+1}) .* (E^T alpha_s) with E = exp(transitions); each step is a
  PE matmul plus one elementwise DVE multiply (PSUM f32 x SBUF bf16).
- The 511-step serial chain is cut ~24x: 32 segments of 16 steps run as
  concurrent chains (one [128,1024] matmul round split in two halves);
  interior segments converge from a uniform vector during 5 burn-in rounds
  (Birkhoff contraction ~0.1/step). Per-segment growth is accounted via
  boundary column sums; fp32 range kept by 2 column rescales folded into
  the round multiply (scalar_tensor_tensor).
- em j-slabs stream via HWDGE DMA in consumption order; ACT exponentiates
  each slab just ahead of the chain. Constants go on the scalar HWDGE ring
  so descriptor generation overlaps.
- Numerator: two indirect-DMA element gathers + reductions, overlapped.

kernel() contract: full unsharded inputs in, full output (scalar) out.
"""
import numpy as np
import ml_dtypes

S, B, T = 512, 256, 128
NCORES, Bl = 8, 32
NSEG, BURN = 32, 5
SEGL = S // NSEG                     # 16 steps per segment
NR = BURN + SEGL                     # 21 rounds
H = NSEG // 2
RESC_APPLY = [BURN + 3, BURN + 9]
C_RESC = 2.0 ** -46                  # constant column rescale factor
RESC_LOGSUM = len(RESC_APPLY) * 46 * float(np.log(2.0))
INIT_BURN = 2.0 ** -30
TSSE_N = T * T + T + T + 1           # 16641: trans | start | end | 0.0
TSSE_PAD = TSSE_N - 1                # index of the 0.0 entry
NCOL = SEGL * NSEG * Bl              # 16384 em columns (j, k, b)
SLAB = NSEG * Bl                     # 1024 columns per j-slab

_NC = None


def _build():
    import concourse.bass as bass
    import concourse.tile as tile
    from concourse import bacc, mybir
    from contextlib import ExitStack

    f32 = mybir.dt.float32
    bf16 = mybir.dt.bfloat16
    i32 = mybir.dt.int32
    AF = mybir.ActivationFunctionType
    OP = mybir.AluOpType
    AX = mybir.AxisListType

    nc = bacc.Bacc("TRN2", target_bir_lowering=False, debug=False,
                   num_devices=NCORES)

    emT = nc.dram_tensor("emT", [T, NCOL], bf16, kind="ExternalInput")
    emg = nc.dram_tensor("emg", [S * Bl, T], f32, kind="ExternalInput")
    transm = nc.dram_tensor("transm", [T, T], f32, kind="ExternalInput")
    startv = nc.dram_tensor("startv", [T, 1], f32, kind="ExternalInput")
    endv = nc.dram_tensor("endv", [T, 1], f32, kind="ExternalInput")
    emtidx = nc.dram_tensor("emtidx", [128, 128], i32, kind="ExternalInput")
    tssev = nc.dram_tensor("tssev", [TSSE_N, 1], f32, kind="ExternalInput")
    tsseidx = nc.dram_tensor("tsseidx", [128, 129], i32, kind="ExternalInput")
    outv = nc.dram_tensor("out", [1, 1], f32, kind="ExternalOutput")

    with tile.TileContext(nc) as tc, ExitStack() as ctx:
        const = ctx.enter_context(tc.tile_pool(name="const", bufs=1))
        pchain = ctx.enter_context(tc.tile_pool(name="pchain", bufs=2,
                                                space="PSUM"))
        pstat = ctx.enter_context(tc.tile_pool(name="pstat", bufs=2,
                                               space="PSUM"))

        # ---------- em slab DMAs first (sync HWDGE ring) ----------
        emsb = const.tile([128, NCOL], bf16)

        def dma_slabs(j0, j1):
            nc.sync.dma_start(out=emsb[:, SLAB * j0:SLAB * j1],
                              in_=emT[:, SLAB * j0:SLAB * j1])

        def dma_slabs_gp(j0, j1):
            nc.gpsimd.dma_start(out=emsb[:, SLAB * j0:SLAB * j1],
                                in_=emT[:, SLAB * j0:SLAB * j1])

        # head slabs singly on the sync ring for earliest landing; steady
        # tail slabs stream in parallel on the gpsimd (SWDGE) ring
        for j in (SEGL - 4, SEGL - 3, SEGL - 2, SEGL - 1, 0, 1, 2, 3):
            dma_slabs(j, j + 1)
        dma_slabs_gp(4, 6)
        dma_slabs_gp(6, 8)
        dma_slabs_gp(8, 10)
        dma_slabs_gp(10, 12)

        # ---------- constants (scalar HWDGE ring, overlapped) ----------
        tr_sb = const.tile([128, 128], f32)
        nc.scalar.dma_start(out=tr_sb[:], in_=transm[:, :])
        st_sb = const.tile([128, 1], f32)
        nc.scalar.dma_start(out=st_sb[:], in_=startv[:, :])
        en_sb = const.tile([128, 1], f32)
        nc.scalar.dma_start(out=en_sb[:], in_=endv[:, :])
        # gather index tiles go last on the sync ring so the scattered
        # gathers don't contend with the head slab DMAs
        emtidx_sb = const.tile([128, 128], i32)
        nc.sync.dma_start(out=emtidx_sb[:], in_=emtidx[:, :])
        tsseidx_sb = const.tile([128, 129], i32)
        nc.sync.dma_start(out=tsseidx_sb[:], in_=tsseidx[:, :])

        ones_col = const.tile([128, 1], bf16)
        nc.vector.memset(ones_col[:], 1.0)
        ones_colf = const.tile([128, 1], f32)
        nc.vector.memset(ones_colf[:], 1.0)

        # ---------- numerator: indirect gathers (gpsimd SWDGE) ----------
        gem = const.tile([128, 128], f32)
        nc.gpsimd.indirect_dma_start(
            out=gem[:], out_offset=None,
            in_=bass.AP(tensor=emg, offset=0,
                        ap=[[1, S * Bl * T], [1, 1]]),
            in_offset=bass.IndirectOffsetOnAxis(ap=emtidx_sb[:], axis=0))
        gts = const.tile([128, 129], f32)
        nc.gpsimd.indirect_dma_start(
            out=gts[:], out_offset=None,
            in_=bass.AP(tensor=tssev, offset=0,
                        ap=[[1, TSSE_N], [1, 1]]),
            in_offset=bass.IndirectOffsetOnAxis(ap=tsseidx_sb[:], axis=0))

        # ---------- chain state ----------
        erm = const.tile([128, NCOL], bf16)
        A = const.tile([128, NSEG, Bl], bf16)
        nc.vector.memset(A[:], INIT_BURN)
        A2 = A.rearrange("p k b -> p (k b)")
        A2snap = const.tile([128, NSEG * Bl], bf16)

        # n_sb/m_sb stored (b, k)-major so the tail reduce streams
        # contiguously over k
        n_sb = const.tile([1, NSEG * Bl], f32)
        m_sb = const.tile([1, NSEG * Bl], f32)
        n_sbv = n_sb.rearrange("p (b k) -> p k b", k=NSEG)
        m_sbv = m_sb.rearrange("p (b k) -> p k b", k=NSEG)
        fin_sb = const.tile([1, Bl], f32)

        def exp_slab(j):
            nc.scalar.activation(erm[:, SLAB * j:SLAB * (j + 1)],
                                 emsb[:, SLAB * j:SLAB * (j + 1)], AF.Exp)

        def erm_off(r, ka):
            # column offset into erm for round r, half starting at segment ka
            if r < BURN - 1:            # replay steps of previous segment
                return (r + SEGL - BURN + 1) * SLAB + (ka - 1) * Bl
            if r == BURN - 1:           # s = SEGL*k rows (j=0, seg k)
                return ka * Bl
            c = r - BURN + 1            # own-segment step 1..SEGL
            if c <= SEGL - 1:
                return c * SLAB + ka * Bl
            return (ka + 1) * Bl        # step SEGL = (j=0, seg k+1)

        def colsum(dstv, klo, khi, src=None):
            # dstv[0, klo:khi, :] = column sums of segments [klo, khi)
            src = A2 if src is None else src
            ps = pstat.tile([1, 512], f32, tag="st")
            w = (khi - klo) * Bl
            nc.tensor.matmul(out=ps[:, :w], lhsT=ones_col[:],
                             rhs=src[:, klo * Bl:khi * Bl],
                             start=True, stop=True)
            nc.vector.tensor_copy(
                out=dstv[:, klo:khi, :],
                in_=ps[:, :w].rearrange("p (k b) -> p k b", b=Bl))

        def emit_round(r):
            if r < BURN:
                ksl = [(1, H), (H, NSEG)]
            elif r < NR - 1:
                ksl = [(0, H), (H, NSEG)]
            else:
                ksl = [(0, H), (H, NSEG - 1)]
            for (ka, kb), tg in zip(ksl, ("psA", "psB")):
                w = (kb - ka) * Bl
                ps = pchain.tile([128, H * Bl], f32, tag=tg)
                nc.tensor.matmul(out=ps[:, :w], lhsT=E_hi[:],
                                 rhs=A2[:, ka * Bl:kb * Bl],
                                 start=True, stop=True)
                off = erm_off(r, ka)
                if r in RESC_APPLY:
                    nc.vector.scalar_tensor_tensor(
                        out=A2[:, ka * Bl:kb * Bl], in0=ps[:, :w],
                        scalar=C_RESC, in1=erm[:, off:off + w],
                        op0=OP.mult, op1=OP.mult)
                else:
                    nc.vector.tensor_tensor(
                        out=A2[:, ka * Bl:kb * Bl], in0=ps[:, :w],
                        in1=erm[:, off:off + w], op=OP.mult)
            if r == BURN - 1:
                # snapshot A2 (cheap 4x bf16 copy); n-colsums happen at the
                # tail so the PE chain keeps E resident
                nc.vector.tensor_copy(out=A2snap[:], in_=A2[:])
            if r == NR - 2:
                m15 = pstat.tile([1, 512], f32, tag="st")
                nc.tensor.matmul(out=m15[:, :Bl], lhsT=ones_col[:],
                                 rhs=A2[:, (NSEG - 1) * Bl:],
                                 start=True, stop=True)
                nc.vector.tensor_copy(out=m_sbv[:, NSEG - 1, :],
                                      in_=m15[:, :Bl])
                fin = pstat.tile([1, 512], f32, tag="st")
                nc.tensor.matmul(out=fin[:, :Bl], lhsT=Eend[:],
                                 rhs=A2[:, (NSEG - 1) * Bl:],
                                 start=True, stop=True)
                nc.vector.tensor_copy(out=fin_sb[:], in_=fin[:, :Bl])
            if r == NR - 1:
                colsum(m_sbv, 0, H)
                colsum(m_sbv, H, NSEG - 1)

        # ---------- emission pipeline + chain ----------
        E_f = const.tile([128, 128], f32)
        nc.scalar.activation(E_f[:], tr_sb[:], AF.Exp)
        E_hi = const.tile([128, 128], bf16)
        nc.vector.tensor_copy(out=E_hi[:], in_=E_f[:])
        Eend = const.tile([128, 1], bf16)
        nc.scalar.activation(Eend[:], en_sb[:], AF.Exp)
        for j in (SEGL - 4, SEGL - 3, SEGL - 2, SEGL - 1, 0):
            exp_slab(j)
        # segment 0 seed: alpha_0 = exp(em_0 + start) via ACT bias (exact)
        nc.scalar.activation(A[:, 0, :], emsb[:, 0:Bl], AF.Exp,
                             bias=st_sb[:])
        for j in range(1, SEGL - 4):
            exp_slab(j)
        for r in range(NR):
            emit_round(r)

        # ---------- final assembly ----------
        colsum(n_sbv, 0, H, src=A2snap)
        colsum(n_sbv, H, NSEG, src=A2snap)
        gsum1 = const.tile([128, 1], f32)
        nc.vector.reduce_sum(out=gsum1[:], in_=gem[:], axis=AX.X)
        gsum2 = const.tile([128, 1], f32)
        nc.vector.reduce_sum(out=gsum2[:], in_=gts[:], axis=AX.X)
        numcol = const.tile([128, 1], f32)
        nc.vector.tensor_add(out=numcol[:], in0=gsum1[:], in1=gsum2[:])
        logn = const.tile([1, NSEG * Bl], f32)
        nc.scalar.activation(logn[:], n_sb[:], AF.Ln)
        logm = const.tile([1, NSEG * Bl], f32)
        nc.scalar.activation(logm[:], m_sb[:], AF.Ln)
        grow = const.tile([1, NSEG * Bl], f32)
        nc.vector.scalar_tensor_tensor(
            out=grow[:], in0=logm[:], scalar=RESC_LOGSUM, in1=logn[:],
            op0=OP.add, op1=OP.subtract)
        growb = const.tile([1, Bl], f32)
        nc.vector.reduce_sum(out=growb[:],
                             in_=grow.rearrange("p (b k) -> p b k", k=NSEG),
                             axis=AX.X)
        logfin = const.tile([1, Bl], f32)
        nc.scalar.activation(logfin[:], fin_sb[:], AF.Ln)
        logm_v = logm.rearrange("p (b k) -> p k b", k=NSEG)
        logn_v = logn.rearrange("p (b k) -> p k b", k=NSEG)
        lz = const.tile([1, Bl], f32)
        nc.vector.tensor_add(out=lz[:], in0=growb[:], in1=logfin[:])
        nc.vector.tensor_tensor(out=lz[:], in0=lz[:],
                                in1=logm_v[:, NSEG - 1, :], op=OP.subtract)
        nc.vector.tensor_add(out=lz[:], in0=lz[:], in1=logn_v[:, 0, :])
        lzs = const.tile([1, 1], f32)
        nc.vector.reduce_sum(out=lzs[:], in_=lz[:], axis=AX.X)
        nps = pstat.tile([1, 512], f32, tag="st")
        nc.tensor.matmul(out=nps[:, :1], lhsT=ones_colf[:], rhs=numcol[:],
                         start=True, stop=True)
        res = const.tile([1, 1], f32)
        nc.vector.tensor_tensor(out=res[:], in0=nps[:, :1], in1=lzs[:],
                                op=OP.subtract)
        nc.sync.dma_start(out=outv[:, :], in_=res[:])

    nc.compile()
    return nc


def _get_nc():
    global _NC
    if _NC is None:
        _NC = _build()
    return _NC


def make_in_maps(inputs):
    em = np.asarray(inputs["emissions"], dtype=np.float32)
    tags = np.asarray(inputs["tags"]).astype(np.int32)
    st = np.asarray(inputs["start_transitions"], dtype=np.float32)
    en = np.asarray(inputs["end_transitions"], dtype=np.float32)
    tr = np.ascontiguousarray(np.asarray(inputs["transitions"],
                                         dtype=np.float32))
    tssev = np.concatenate(
        [tr.ravel(), st, en, np.zeros(1, np.float32)]).astype(
        np.float32).reshape(TSSE_N, 1)
    s_i = np.arange(S)[:, None]
    b_i = np.arange(Bl)[None, :]
    in_maps = []
    for c in range(NCORES):
        em_c = em[:, c * Bl:(c + 1) * Bl, :]
        # [s=(k,j), b, t] -> [t, j, k, b]; col = j*SLAB + k*Bl + b
        emr = np.ascontiguousarray(
            em_c.reshape(NSEG, SEGL, Bl, T).transpose(3, 1, 0, 2)
        ).reshape(T, NCOL)
        emT_b = emr.astype(ml_dtypes.bfloat16)
        tg = tags[:, c * Bl:(c + 1) * Bl]
        emi = ((s_i * Bl + b_i) * T + tg).astype(np.int32).reshape(128, 128)
        tse = np.full(128 * 129, TSSE_PAD, np.int32)
        tse[:511 * Bl] = (tg[:-1] * T + tg[1:]).astype(np.int32).ravel()
        tse[511 * Bl:511 * Bl + Bl] = T * T + tg[0]
        tse[511 * Bl + Bl:511 * Bl + 2 * Bl] = T * T + T + tg[-1]
        in_maps.append({
            "emT": emT_b,
            "emg": np.ascontiguousarray(em_c).reshape(S * Bl, T),
            "transm": tr,
            "startv": st.reshape(T, 1),
            "endv": en.reshape(T, 1),
            "emtidx": emi,
            "tssev": tssev,
            "tsseidx": tse.reshape(128, 129),
        })
    return in_maps


def _numpy_fallback(inputs):
    """Exact float64 port of the reference (handles arbitrary masks)."""
    em = np.asarray(inputs["emissions"], dtype=np.float64)
    tags = np.asarray(inputs["tags"]).astype(np.int64)
    mask = np.asarray(inputs["mask"]).astype(bool)
    st = np.asarray(inputs["start_transitions"], dtype=np.float64)
    en = np.asarray(inputs["end_transitions"], dtype=np.float64)
    tr = np.asarray(inputs["transitions"], dtype=np.float64)
    Sl, Bn = tags.shape
    mask_f = mask.astype(np.float64)
    emit = np.take_along_axis(em, tags[:, :, None], axis=2)[:, :, 0]
    trsc = tr[tags[:-1], tags[1:]]
    score = st[tags[0]] + emit[0]
    score = score + ((trsc + emit[1:]) * mask_f[1:]).sum(0)
    seq_ends = mask.astype(np.int64).sum(0) - 1
    score = score + en[tags[seq_ends, np.arange(Bn)]]
    alpha = st[None, :] + em[0]
    for s in range(1, Sl):
        nxt = alpha[:, :, None] + tr[None] + em[s][:, None, :]
        mx = nxt.max(axis=1)
        nxt = mx + np.log(np.exp(nxt - mx[:, None, :]).sum(axis=1))
        alpha = np.where(mask[s][:, None], nxt, alpha)
    z = alpha + en[None, :]
    mz = z.max(axis=1)
    logZ = mz + np.log(np.exp(z - mz[:, None]).sum(axis=1))
    return np.asarray((score - logZ).sum(), dtype=np.float32)


def run_device(inputs, trace=False, trace_kwargs=None):
    from concourse.bass_utils import run_bass_kernel_spmd
    nc = _get_nc()
    in_maps = make_in_maps(inputs)
    br = run_bass_kernel_spmd(nc, in_maps, list(range(NCORES)),
                              trace=trace, **(trace_kwargs or {}))
    total = np.float32(
        sum(float(br.results[i]["out"][0, 0]) for i in range(NCORES)))
    return np.asarray(total, dtype=np.float32), br


def kernel(**inputs):
    mask = np.asarray(inputs["mask"])
    if not bool(mask.all()):
        return _numpy_fallback(inputs)
    val, _ = run_device(inputs, trace=False)
    return val


# revision 26
# speedup vs baseline: 1.1133x; 1.1133x over previous
"""Trainium2 Bass kernel for the BiLSTM-CRF loss (sum reduction).

Strategy (v4):
- Data-parallel: batch 256 sharded as 32 per NeuronCore across 8 cores.
- Host passes emissions pre-transposed to [T, (step j, seg k, batch b)] and
  pre-cast to bf16 (layout/dtype prep only; all math stays on device), plus
  a f32 copy for the exact numerator gather. This removes all on-device PE
  transposes and DVE casts and halves streamed HBM traffic.
- Normalizer (forward algorithm) runs in LINEAR space: alpha_{s+1} =
  exp(em_{s+1}) .* (E^T alpha_s) with E = exp(transitions); each step is a
  PE matmul plus one elementwise DVE multiply (PSUM f32 x SBUF bf16).
- The 511-step serial chain is cut ~24x: 32 segments of 16 steps run as
  concurrent chains (one [128,1024] matmul round split in two halves);
  interior segments converge from a uniform vector during 5 burn-in rounds
  (Birkhoff contraction ~0.1/step). Per-segment growth is accounted via
  boundary column sums; fp32 range kept by 2 column rescales folded into
  the round multiply (scalar_tensor_tensor).
- em j-slabs stream via HWDGE DMA in consumption order; ACT exponentiates
  each slab just ahead of the chain. Constants go on the scalar HWDGE ring
  so descriptor generation overlaps.
- Numerator: two indirect-DMA element gathers + reductions, overlapped.

kernel() contract: full unsharded inputs in, full output (scalar) out.
"""
import numpy as np
import ml_dtypes

S, B, T = 512, 256, 128
NCORES, Bl = 8, 32
NSEG, BURN = 32, 4
SEGL = S // NSEG                     # 16 steps per segment
NR = BURN + SEGL                     # 21 rounds
H = NSEG // 2
RESC_APPLY = [BURN + 3, BURN + 9]
C_RESC = 2.0 ** -46                  # constant column rescale factor
RESC_LOGSUM = len(RESC_APPLY) * 46 * float(np.log(2.0))
INIT_BURN = 2.0 ** -30
TSSE_N = T * T + T + T + 1           # 16641: trans | start | end | 0.0
TSSE_PAD = TSSE_N - 1                # index of the 0.0 entry
NCOL = SEGL * NSEG * Bl              # 16384 em columns (j, k, b)
SLAB = NSEG * Bl                     # 1024 columns per j-slab

_NC = None


def _build():
    import concourse.bass as bass
    import concourse.tile as tile
    from concourse import bacc, mybir
    from contextlib import ExitStack

    f32 = mybir.dt.float32
    bf16 = mybir.dt.bfloat16
    i32 = mybir.dt.int32
    AF = mybir.ActivationFunctionType
    OP = mybir.AluOpType
    AX = mybir.AxisListType

    nc = bacc.Bacc("TRN2", target_bir_lowering=False, debug=False,
                   num_devices=NCORES)

    emT = nc.dram_tensor("emT", [T, NCOL], bf16, kind="ExternalInput")
    emg = nc.dram_tensor("emg", [S * Bl, T], f32, kind="ExternalInput")
    transm = nc.dram_tensor("transm", [T, T], f32, kind="ExternalInput")
    startv = nc.dram_tensor("startv", [T, 1], f32, kind="ExternalInput")
    endv = nc.dram_tensor("endv", [T, 1], f32, kind="ExternalInput")
    emtidx = nc.dram_tensor("emtidx", [128, 128], i32, kind="ExternalInput")
    tssev = nc.dram_tensor("tssev", [TSSE_N, 1], f32, kind="ExternalInput")
    tsseidx = nc.dram_tensor("tsseidx", [128, 129], i32, kind="ExternalInput")
    outv = nc.dram_tensor("out", [1, 1], f32, kind="ExternalOutput")

    with tile.TileContext(nc) as tc, ExitStack() as ctx:
        const = ctx.enter_context(tc.tile_pool(name="const", bufs=1))
        pchain = ctx.enter_context(tc.tile_pool(name="pchain", bufs=2,
                                                space="PSUM"))
        pstat = ctx.enter_context(tc.tile_pool(name="pstat", bufs=2,
                                               space="PSUM"))

        # ---------- em slab DMAs first (sync HWDGE ring) ----------
        emsb = const.tile([128, NCOL], bf16)

        def dma_slabs(j0, j1):
            nc.sync.dma_start(out=emsb[:, SLAB * j0:SLAB * j1],
                              in_=emT[:, SLAB * j0:SLAB * j1])

        # all slabs on the sync ring, in consumption order: head slabs
        # singly for earliest landing, steady tail as pairs
        for j in (SEGL - 3, SEGL - 2, SEGL - 1, 0, 1, 2, 3):
            dma_slabs(j, j + 1)
        dma_slabs(4, 6)
        dma_slabs(6, 8)
        dma_slabs(8, 10)
        dma_slabs(10, 12)
        dma_slabs(SEGL - 4, SEGL - 3)

        # ---------- constants (scalar HWDGE ring, overlapped) ----------
        tr_sb = const.tile([128, 128], f32)
        nc.scalar.dma_start(out=tr_sb[:], in_=transm[:, :])
        st_sb = const.tile([128, 1], f32)
        nc.scalar.dma_start(out=st_sb[:], in_=startv[:, :])
        en_sb = const.tile([128, 1], f32)
        nc.scalar.dma_start(out=en_sb[:], in_=endv[:, :])
        # gather index tiles go last on the sync ring so the scattered
        # gathers don't contend with the head slab DMAs
        emtidx_sb = const.tile([128, 128], i32)
        nc.sync.dma_start(out=emtidx_sb[:], in_=emtidx[:, :])
        tsseidx_sb = const.tile([128, 129], i32)
        nc.sync.dma_start(out=tsseidx_sb[:], in_=tsseidx[:, :])

        ones_col = const.tile([128, 1], bf16)
        nc.vector.memset(ones_col[:], 1.0)
        ones_colf = const.tile([128, 1], f32)
        nc.vector.memset(ones_colf[:], 1.0)

        # ---------- numerator: indirect gathers (gpsimd SWDGE) ----------
        gem = const.tile([128, 128], f32)
        nc.gpsimd.indirect_dma_start(
            out=gem[:], out_offset=None,
            in_=bass.AP(tensor=emg, offset=0,
                        ap=[[1, S * Bl * T], [1, 1]]),
            in_offset=bass.IndirectOffsetOnAxis(ap=emtidx_sb[:], axis=0))
        gts = const.tile([128, 129], f32)
        nc.gpsimd.indirect_dma_start(
            out=gts[:], out_offset=None,
            in_=bass.AP(tensor=tssev, offset=0,
                        ap=[[1, TSSE_N], [1, 1]]),
            in_offset=bass.IndirectOffsetOnAxis(ap=tsseidx_sb[:], axis=0))

        # ---------- chain state ----------
        erm = const.tile([128, NCOL], bf16)
        A = const.tile([128, NSEG, Bl], bf16)
        nc.vector.memset(A[:], INIT_BURN)
        A2 = A.rearrange("p k b -> p (k b)")
        A2snap = const.tile([128, NSEG * Bl], bf16)

        # n_sb/m_sb stored (b, k)-major so the tail reduce streams
        # contiguously over k
        n_sb = const.tile([1, NSEG * Bl], f32)
        m_sb = const.tile([1, NSEG * Bl], f32)
        n_sbv = n_sb.rearrange("p (b k) -> p k b", k=NSEG)
        m_sbv = m_sb.rearrange("p (b k) -> p k b", k=NSEG)
        fin_sb = const.tile([1, Bl], f32)

        def exp_slab(j):
            nc.scalar.activation(erm[:, SLAB * j:SLAB * (j + 1)],
                                 emsb[:, SLAB * j:SLAB * (j + 1)], AF.Exp)

        def erm_off(r, ka):
            # column offset into erm for round r, half starting at segment ka
            if r < BURN - 1:            # replay steps of previous segment
                return (r + SEGL - BURN + 1) * SLAB + (ka - 1) * Bl
            if r == BURN - 1:           # s = SEGL*k rows (j=0, seg k)
                return ka * Bl
            c = r - BURN + 1            # own-segment step 1..SEGL
            if c <= SEGL - 1:
                return c * SLAB + ka * Bl
            return (ka + 1) * Bl        # step SEGL = (j=0, seg k+1)

        def colsum(dstv, klo, khi, src=None):
            # dstv[0, klo:khi, :] = column sums of segments [klo, khi)
            src = A2 if src is None else src
            ps = pstat.tile([1, 512], f32, tag="st")
            w = (khi - klo) * Bl
            nc.tensor.matmul(out=ps[:, :w], lhsT=ones_col[:],
                             rhs=src[:, klo * Bl:khi * Bl],
                             start=True, stop=True)
            nc.vector.tensor_copy(
                out=dstv[:, klo:khi, :],
                in_=ps[:, :w].rearrange("p (k b) -> p k b", b=Bl))

        def emit_round(r):
            if r < BURN:
                ksl = [(1, H), (H, NSEG)]
            elif r < NR - 1:
                ksl = [(0, H), (H, NSEG)]
            else:
                ksl = [(0, H), (H, NSEG - 1)]
            for (ka, kb), tg in zip(ksl, ("psA", "psB")):
                w = (kb - ka) * Bl
                ps = pchain.tile([128, H * Bl], f32, tag=tg)
                nc.tensor.matmul(out=ps[:, :w], lhsT=E_hi[:],
                                 rhs=A2[:, ka * Bl:kb * Bl],
                                 start=True, stop=True)
                off = erm_off(r, ka)
                if r in RESC_APPLY:
                    nc.vector.scalar_tensor_tensor(
                        out=A2[:, ka * Bl:kb * Bl], in0=ps[:, :w],
                        scalar=C_RESC, in1=erm[:, off:off + w],
                        op0=OP.mult, op1=OP.mult)
                else:
                    nc.vector.tensor_tensor(
                        out=A2[:, ka * Bl:kb * Bl], in0=ps[:, :w],
                        in1=erm[:, off:off + w], op=OP.mult)
            if r == BURN - 1:
                # snapshot A2 (cheap 4x bf16 copy); n-colsums happen at the
                # tail so the PE chain keeps E resident
                nc.vector.tensor_copy(out=A2snap[:], in_=A2[:])
            if r == NR - 2:
                m15 = pstat.tile([1, 512], f32, tag="st")
                nc.tensor.matmul(out=m15[:, :Bl], lhsT=ones_col[:],
                                 rhs=A2[:, (NSEG - 1) * Bl:],
                                 start=True, stop=True)
                nc.vector.tensor_copy(out=m_sbv[:, NSEG - 1, :],
                                      in_=m15[:, :Bl])
                fin = pstat.tile([1, 512], f32, tag="st")
                nc.tensor.matmul(out=fin[:, :Bl], lhsT=Eend[:],
                                 rhs=A2[:, (NSEG - 1) * Bl:],
                                 start=True, stop=True)
                nc.vector.tensor_copy(out=fin_sb[:], in_=fin[:, :Bl])
            if r == NR - 1:
                colsum(m_sbv, 0, H)
                colsum(m_sbv, H, NSEG - 1)

        # ---------- emission pipeline + chain ----------
        E_f = const.tile([128, 128], f32)
        nc.scalar.activation(E_f[:], tr_sb[:], AF.Exp)
        E_hi = const.tile([128, 128], bf16)
        nc.vector.tensor_copy(out=E_hi[:], in_=E_f[:])
        Eend = const.tile([128, 1], bf16)
        nc.scalar.activation(Eend[:], en_sb[:], AF.Exp)
        for j in list(range(SEGL - BURN + 1, SEGL)) + [0]:
            exp_slab(j)
        # segment 0 seed: alpha_0 = exp(em_0 + start) via ACT bias (exact)
        nc.scalar.activation(A[:, 0, :], emsb[:, 0:Bl], AF.Exp,
                             bias=st_sb[:])
        for j in range(1, SEGL - BURN + 1):
            exp_slab(j)
        for r in range(NR):
            emit_round(r)
            if r == BURN + 3:
                colsum(n_sbv, 0, H, src=A2snap)
            if r == BURN + 5:
                colsum(n_sbv, H, NSEG, src=A2snap)

        # ---------- final assembly ----------
        gsum1 = const.tile([128, 1], f32)
        nc.vector.reduce_sum(out=gsum1[:], in_=gem[:], axis=AX.X)
        gsum2 = const.tile([128, 1], f32)
        nc.vector.reduce_sum(out=gsum2[:], in_=gts[:], axis=AX.X)
        numcol = const.tile([128, 1], f32)
        nc.vector.tensor_add(out=numcol[:], in0=gsum1[:], in1=gsum2[:])
        logn = const.tile([1, NSEG * Bl], f32)
        nc.scalar.activation(logn[:], n_sb[:], AF.Ln)
        logm = const.tile([1, NSEG * Bl], f32)
        nc.scalar.activation(logm[:], m_sb[:], AF.Ln)
        grow = const.tile([1, NSEG * Bl], f32)
        nc.vector.scalar_tensor_tensor(
            out=grow[:], in0=logm[:], scalar=RESC_LOGSUM, in1=logn[:],
            op0=OP.add, op1=OP.subtract)
        growb = const.tile([1, Bl], f32)
        nc.vector.reduce_sum(out=growb[:],
                             in_=grow.rearrange("p (b k) -> p b k", k=NSEG),
                             axis=AX.X)
        logfin = const.tile([1, Bl], f32)
        nc.scalar.activation(logfin[:], fin_sb[:], AF.Ln)
        logm_v = logm.rearrange("p (b k) -> p k b", k=NSEG)
        logn_v = logn.rearrange("p (b k) -> p k b", k=NSEG)
        lz = const.tile([1, Bl], f32)
        nc.vector.tensor_add(out=lz[:], in0=growb[:], in1=logfin[:])
        nc.vector.tensor_tensor(out=lz[:], in0=lz[:],
                                in1=logm_v[:, NSEG - 1, :], op=OP.subtract)
        nc.vector.tensor_add(out=lz[:], in0=lz[:], in1=logn_v[:, 0, :])
        lzs = const.tile([1, 1], f32)
        nc.vector.reduce_sum(out=lzs[:], in_=lz[:], axis=AX.X)
        nps = pstat.tile([1, 512], f32, tag="st")
        nc.tensor.matmul(out=nps[:, :1], lhsT=ones_colf[:], rhs=numcol[:],
                         start=True, stop=True)
        res = const.tile([1, 1], f32)
        nc.vector.tensor_tensor(out=res[:], in0=nps[:, :1], in1=lzs[:],
                                op=OP.subtract)
        nc.sync.dma_start(out=outv[:, :], in_=res[:])

    nc.compile()
    return nc


def _get_nc():
    global _NC
    if _NC is None:
        _NC = _build()
    return _NC


def make_in_maps(inputs):
    em = np.asarray(inputs["emissions"], dtype=np.float32)
    tags = np.asarray(inputs["tags"]).astype(np.int32)
    st = np.asarray(inputs["start_transitions"], dtype=np.float32)
    en = np.asarray(inputs["end_transitions"], dtype=np.float32)
    tr = np.ascontiguousarray(np.asarray(inputs["transitions"],
                                         dtype=np.float32))
    tssev = np.concatenate(
        [tr.ravel(), st, en, np.zeros(1, np.float32)]).astype(
        np.float32).reshape(TSSE_N, 1)
    s_i = np.arange(S)[:, None]
    b_i = np.arange(Bl)[None, :]
    in_maps = []
    for c in range(NCORES):
        em_c = em[:, c * Bl:(c + 1) * Bl, :]
        # [s=(k,j), b, t] -> [t, j, k, b]; col = j*SLAB + k*Bl + b
        emr = np.ascontiguousarray(
            em_c.reshape(NSEG, SEGL, Bl, T).transpose(3, 1, 0, 2)
        ).reshape(T, NCOL)
        emT_b = emr.astype(ml_dtypes.bfloat16)
        tg = tags[:, c * Bl:(c + 1) * Bl]
        emi = ((s_i * Bl + b_i) * T + tg).astype(np.int32).reshape(128, 128)
        tse = np.full(128 * 129, TSSE_PAD, np.int32)
        tse[:511 * Bl] = (tg[:-1] * T + tg[1:]).astype(np.int32).ravel()
        tse[511 * Bl:511 * Bl + Bl] = T * T + tg[0]
        tse[511 * Bl + Bl:511 * Bl + 2 * Bl] = T * T + T + tg[-1]
        in_maps.append({
            "emT": emT_b,
            "emg": np.ascontiguousarray(em_c).reshape(S * Bl, T),
            "transm": tr,
            "startv": st.reshape(T, 1),
            "endv": en.reshape(T, 1),
            "emtidx": emi,
            "tssev": tssev,
            "tsseidx": tse.reshape(128, 129),
        })
    return in_maps


def _numpy_fallback(inputs):
    """Exact float64 port of the reference (handles arbitrary masks)."""
    em = np.asarray(inputs["emissions"], dtype=np.float64)
    tags = np.asarray(inputs["tags"]).astype(np.int64)
    mask = np.asarray(inputs["mask"]).astype(bool)
    st = np.asarray(inputs["start_transitions"], dtype=np.float64)
    en = np.asarray(inputs["end_transitions"], dtype=np.float64)
    tr = np.asarray(inputs["transitions"], dtype=np.float64)
    Sl, Bn = tags.shape
    mask_f = mask.astype(np.float64)
    emit = np.take_along_axis(em, tags[:, :, None], axis=2)[:, :, 0]
    trsc = tr[tags[:-1], tags[1:]]
    score = st[tags[0]] + emit[0]
    score = score + ((trsc + emit[1:]) * mask_f[1:]).sum(0)
    seq_ends = mask.astype(np.int64).sum(0) - 1
    score = score + en[tags[seq_ends, np.arange(Bn)]]
    alpha = st[None, :] + em[0]
    for s in range(1, Sl):
        nxt = alpha[:, :, None] + tr[None] + em[s][:, None, :]
        mx = nxt.max(axis=1)
        nxt = mx + np.log(np.exp(nxt - mx[:, None, :]).sum(axis=1))
        alpha = np.where(mask[s][:, None], nxt, alpha)
    z = alpha + en[None, :]
    mz = z.max(axis=1)
    logZ = mz + np.log(np.exp(z - mz[:, None]).sum(axis=1))
    return np.asarray((score - logZ).sum(), dtype=np.float32)


def run_device(inputs, trace=False, trace_kwargs=None):
    from concourse.bass_utils import run_bass_kernel_spmd
    nc = _get_nc()
    in_maps = make_in_maps(inputs)
    br = run_bass_kernel_spmd(nc, in_maps, list(range(NCORES)),
                              trace=trace, **(trace_kwargs or {}))
    total = np.float32(
        sum(float(br.results[i]["out"][0, 0]) for i in range(NCORES)))
    return np.asarray(total, dtype=np.float32), br


def kernel(**inputs):
    mask = np.asarray(inputs["mask"])
    if not bool(mask.all()):
        return _numpy_fallback(inputs)
    val, _ = run_device(inputs, trace=False)
    return val


# revision 30
# speedup vs baseline: 1.1623x; 1.0440x over previous
"""Trainium2 Bass kernel for the BiLSTM-CRF loss (sum reduction).

Strategy (v4):
- Data-parallel: batch 256 sharded as 32 per NeuronCore across 8 cores.
- Host passes emissions pre-transposed to [T, (step j, seg k, batch b)] and
  pre-cast to bf16 (layout/dtype prep only; all math stays on device), plus
  a f32 copy for the exact numerator gather. This removes all on-device PE
  transposes and DVE casts and halves streamed HBM traffic.
- Normalizer (forward algorithm) runs in LINEAR space: alpha_{s+1} =
  exp(em_{s+1}) .* (E^T alpha_s) with E = exp(transitions); each step is a
  PE matmul plus one elementwise DVE multiply (PSUM f32 x SBUF bf16).
- The 511-step serial chain is cut ~24x: 32 segments of 16 steps run as
  concurrent chains (one [128,1024] matmul round split in two halves);
  interior segments converge from a uniform vector during 5 burn-in rounds
  (Birkhoff contraction ~0.1/step). Per-segment growth is accounted via
  boundary column sums; fp32 range kept by 2 column rescales folded into
  the round multiply (scalar_tensor_tensor).
- em j-slabs stream via HWDGE DMA in consumption order; ACT exponentiates
  each slab just ahead of the chain. Constants go on the scalar HWDGE ring
  so descriptor generation overlaps.
- Numerator: two indirect-DMA element gathers + reductions, overlapped.

kernel() contract: full unsharded inputs in, full output (scalar) out.
"""
import numpy as np
import ml_dtypes

S, B, T = 512, 256, 128
NCORES, Bl = 8, 32
NSEG, BURN = 32, 4
SEGL = S // NSEG                     # 16 steps per segment
NR = BURN + SEGL                     # 21 rounds
H = NSEG // 2
RESC_APPLY = [BURN + 3, BURN + 9]
C_RESC = 2.0 ** -46                  # constant column rescale factor
RESC_LOGSUM = len(RESC_APPLY) * 46 * float(np.log(2.0))
INIT_BURN = 2.0 ** -30
TSSE_N = T * T + T + T + 1           # 16641: trans | start | end | 0.0
TSSE_PAD = TSSE_N - 1                # index of the 0.0 entry
NCOL = SEGL * NSEG * Bl              # 16384 em columns (j, k, b)
SLAB = NSEG * Bl                     # 1024 columns per j-slab

_NC = None


def _build():
    import concourse.bass as bass
    import concourse.tile as tile
    from concourse import bacc, mybir
    from contextlib import ExitStack

    f32 = mybir.dt.float32
    bf16 = mybir.dt.bfloat16
    i32 = mybir.dt.int32
    AF = mybir.ActivationFunctionType
    OP = mybir.AluOpType
    AX = mybir.AxisListType

    nc = bacc.Bacc("TRN2", target_bir_lowering=False, debug=False,
                   num_devices=NCORES)

    emT = nc.dram_tensor("emT", [T, NCOL], bf16, kind="ExternalInput")
    emg = nc.dram_tensor("emg", [S * Bl, T], f32, kind="ExternalInput")
    transm = nc.dram_tensor("transm", [T, T], f32, kind="ExternalInput")
    startv = nc.dram_tensor("startv", [T, 1], f32, kind="ExternalInput")
    endv = nc.dram_tensor("endv", [T, 1], f32, kind="ExternalInput")
    emtidx = nc.dram_tensor("emtidx", [128, 128], i32, kind="ExternalInput")
    tssev = nc.dram_tensor("tssev", [TSSE_N, 1], f32, kind="ExternalInput")
    tsseidx = nc.dram_tensor("tsseidx", [128, 129], i32, kind="ExternalInput")
    outv = nc.dram_tensor("out", [1, 1], f32, kind="ExternalOutput")

    with tile.TileContext(nc) as tc, ExitStack() as ctx:
        const = ctx.enter_context(tc.tile_pool(name="const", bufs=1))
        pchain = ctx.enter_context(tc.tile_pool(name="pchain", bufs=2,
                                                space="PSUM"))
        pstat = ctx.enter_context(tc.tile_pool(name="pstat", bufs=2,
                                               space="PSUM"))

        # ---------- em slab DMAs first (sync HWDGE ring) ----------
        emsb = const.tile([128, NCOL], bf16)

        def dma_slabs(j0, j1):
            nc.sync.dma_start(out=emsb[:, SLAB * j0:SLAB * j1],
                              in_=emT[:, SLAB * j0:SLAB * j1])

        # all slabs on the sync ring, singly, in consumption order; first
        # slab in halves so the first burn round starts earliest
        nc.sync.dma_start(out=emsb[:, SLAB * (SEGL - 3):SLAB * (SEGL - 3) + 512],
                          in_=emT[:, SLAB * (SEGL - 3):SLAB * (SEGL - 3) + 512])
        nc.sync.dma_start(out=emsb[:, SLAB * (SEGL - 3) + 512:SLAB * (SEGL - 2)],
                          in_=emT[:, SLAB * (SEGL - 3) + 512:SLAB * (SEGL - 2)])
        for j in (SEGL - 2, SEGL - 1, 0, 1, 2, 3, 4, 5, 6, 7, 8, 9, 10, 11):
            dma_slabs(j, j + 1)
        dma_slabs(SEGL - 4, SEGL - 3)

        # ---------- constants (scalar HWDGE ring, overlapped) ----------
        tr_sb = const.tile([128, 128], f32)
        nc.scalar.dma_start(out=tr_sb[:], in_=transm[:, :])
        st_sb = const.tile([128, 1], f32)
        nc.scalar.dma_start(out=st_sb[:], in_=startv[:, :])
        en_sb = const.tile([128, 1], f32)
        nc.scalar.dma_start(out=en_sb[:], in_=endv[:, :])
        # gather index tiles go last on the sync ring so the scattered
        # gathers don't contend with the head slab DMAs
        emtidx_sb = const.tile([128, 128], i32)
        nc.sync.dma_start(out=emtidx_sb[:], in_=emtidx[:, :])
        tsseidx_sb = const.tile([128, 129], i32)
        nc.sync.dma_start(out=tsseidx_sb[:], in_=tsseidx[:, :])

        ones_col = const.tile([128, 1], bf16)
        nc.vector.memset(ones_col[:], 1.0)
        ones_colf = const.tile([128, 1], f32)
        nc.vector.memset(ones_colf[:], 1.0)

        # ---------- numerator: indirect gathers (gpsimd SWDGE) ----------
        gem = const.tile([128, 128], f32)
        nc.gpsimd.indirect_dma_start(
            out=gem[:], out_offset=None,
            in_=bass.AP(tensor=emg, offset=0,
                        ap=[[1, S * Bl * T], [1, 1]]),
            in_offset=bass.IndirectOffsetOnAxis(ap=emtidx_sb[:], axis=0))
        gts = const.tile([128, 129], f32)
        nc.gpsimd.indirect_dma_start(
            out=gts[:], out_offset=None,
            in_=bass.AP(tensor=tssev, offset=0,
                        ap=[[1, TSSE_N], [1, 1]]),
            in_offset=bass.IndirectOffsetOnAxis(ap=tsseidx_sb[:], axis=0))

        # ---------- chain state ----------
        erm = const.tile([128, NCOL], bf16)
        A = const.tile([128, NSEG, Bl], bf16)
        nc.vector.memset(A[:], INIT_BURN)
        A2 = A.rearrange("p k b -> p (k b)")
        A2snap = const.tile([128, NSEG * Bl], bf16)

        # n_sb/m_sb stored (b, k)-major so the tail reduce streams
        # contiguously over k
        n_sb = const.tile([1, NSEG * Bl], f32)
        m_sb = const.tile([1, NSEG * Bl], f32)
        n_sbv = n_sb.rearrange("p (b k) -> p k b", k=NSEG)
        m_sbv = m_sb.rearrange("p (b k) -> p k b", k=NSEG)
        fin_sb = const.tile([1, Bl], f32)

        def exp_slab(j):
            nc.scalar.activation(erm[:, SLAB * j:SLAB * (j + 1)],
                                 emsb[:, SLAB * j:SLAB * (j + 1)], AF.Exp)

        def erm_off(r, ka):
            # column offset into erm for round r, half starting at segment ka
            if r < BURN - 1:            # replay steps of previous segment
                return (r + SEGL - BURN + 1) * SLAB + (ka - 1) * Bl
            if r == BURN - 1:           # s = SEGL*k rows (j=0, seg k)
                return ka * Bl
            c = r - BURN + 1            # own-segment step 1..SEGL
            if c <= SEGL - 1:
                return c * SLAB + ka * Bl
            return (ka + 1) * Bl        # step SEGL = (j=0, seg k+1)

        def colsum(dstv, klo, khi, src=None):
            # dstv[0, klo:khi, :] = column sums of segments [klo, khi)
            src = A2 if src is None else src
            ps = pstat.tile([1, 512], f32, tag="st")
            w = (khi - klo) * Bl
            nc.tensor.matmul(out=ps[:, :w], lhsT=ones_col[:],
                             rhs=src[:, klo * Bl:khi * Bl],
                             start=True, stop=True)
            nc.vector.tensor_copy(
                out=dstv[:, klo:khi, :],
                in_=ps[:, :w].rearrange("p (k b) -> p k b", b=Bl))

        def emit_round(r):
            if r < BURN:
                ksl = [(1, H), (H, NSEG)]
            elif r < NR - 1:
                ksl = [(0, H), (H, NSEG)]
            else:
                ksl = [(0, H), (H, NSEG - 1)]
            for (ka, kb), tg in zip(ksl, ("psA", "psB")):
                w = (kb - ka) * Bl
                ps = pchain.tile([128, H * Bl], f32, tag=tg)
                nc.tensor.matmul(out=ps[:, :w], lhsT=E_hi[:],
                                 rhs=A2[:, ka * Bl:kb * Bl],
                                 start=True, stop=True)
                off = erm_off(r, ka)
                if r in RESC_APPLY:
                    nc.vector.scalar_tensor_tensor(
                        out=A2[:, ka * Bl:kb * Bl], in0=ps[:, :w],
                        scalar=C_RESC, in1=erm[:, off:off + w],
                        op0=OP.mult, op1=OP.mult)
                else:
                    nc.vector.tensor_tensor(
                        out=A2[:, ka * Bl:kb * Bl], in0=ps[:, :w],
                        in1=erm[:, off:off + w], op=OP.mult)
            if r == BURN - 1:
                # snapshot A2 (cheap 4x bf16 copy); n-colsums happen at the
                # tail so the PE chain keeps E resident
                nc.vector.tensor_copy(out=A2snap[:], in_=A2[:])
            if r == NR - 2:
                m15 = pstat.tile([1, 512], f32, tag="st")
                nc.tensor.matmul(out=m15[:, :Bl], lhsT=ones_col[:],
                                 rhs=A2[:, (NSEG - 1) * Bl:],
                                 start=True, stop=True)
                nc.vector.tensor_copy(out=m_sbv[:, NSEG - 1, :],
                                      in_=m15[:, :Bl])
                fin = pstat.tile([1, 512], f32, tag="st")
                nc.tensor.matmul(out=fin[:, :Bl], lhsT=Eend[:],
                                 rhs=A2[:, (NSEG - 1) * Bl:],
                                 start=True, stop=True)
                nc.vector.tensor_copy(out=fin_sb[:], in_=fin[:, :Bl])
            if r == NR - 1:
                # split the two stats copies across ACT and DVE
                psm = pstat.tile([1, 512], f32, tag="st")
                nc.tensor.matmul(out=psm[:], lhsT=ones_col[:],
                                 rhs=A2[:, :H * Bl], start=True, stop=True)
                nc.scalar.copy(
                    out=m_sbv[:, 0:H, :],
                    in_=psm[:].rearrange("p (k b) -> p k b", b=Bl))
                colsum(m_sbv, H, NSEG - 1)

        # ---------- emission pipeline + chain ----------
        E_f = const.tile([128, 128], f32)
        nc.scalar.activation(E_f[:], tr_sb[:], AF.Exp)
        E_hi = const.tile([128, 128], bf16)
        nc.vector.tensor_copy(out=E_hi[:], in_=E_f[:])
        Eend = const.tile([128, 1], bf16)
        nc.scalar.activation(Eend[:], en_sb[:], AF.Exp)
        j0 = SEGL - BURN + 1
        nc.scalar.activation(erm[:, SLAB * j0:SLAB * j0 + 512],
                             emsb[:, SLAB * j0:SLAB * j0 + 512], AF.Exp)
        nc.scalar.activation(erm[:, SLAB * j0 + 512:SLAB * (j0 + 1)],
                             emsb[:, SLAB * j0 + 512:SLAB * (j0 + 1)], AF.Exp)
        for j in list(range(SEGL - BURN + 2, SEGL)) + [0]:
            exp_slab(j)
        # segment 0 seed: alpha_0 = exp(em_0 + start) via ACT bias (exact)
        nc.scalar.activation(A[:, 0, :], emsb[:, 0:Bl], AF.Exp,
                             bias=st_sb[:])
        for j in range(1, SEGL - BURN + 1):
            exp_slab(j)
        for r in range(NR):
            emit_round(r)
            if r == BURN + 3:
                colsum(n_sbv, 0, H, src=A2snap)
            if r == BURN + 5:
                colsum(n_sbv, H, NSEG, src=A2snap)

        # ---------- final assembly ----------
        gsum1 = const.tile([128, 1], f32)
        nc.vector.reduce_sum(out=gsum1[:], in_=gem[:], axis=AX.X)
        gsum2 = const.tile([128, 1], f32)
        nc.vector.reduce_sum(out=gsum2[:], in_=gts[:], axis=AX.X)
        numcol = const.tile([128, 1], f32)
        nc.vector.tensor_add(out=numcol[:], in0=gsum1[:], in1=gsum2[:])
        logn = const.tile([1, NSEG * Bl], f32)
        nc.scalar.activation(logn[:], n_sb[:], AF.Ln)
        logm = const.tile([1, NSEG * Bl], f32)
        nc.scalar.activation(logm[:], m_sb[:], AF.Ln)
        grow = const.tile([1, NSEG * Bl], f32)
        nc.vector.scalar_tensor_tensor(
            out=grow[:], in0=logm[:], scalar=RESC_LOGSUM, in1=logn[:],
            op0=OP.add, op1=OP.subtract)
        growb = const.tile([1, Bl], f32)
        nc.vector.reduce_sum(out=growb[:],
                             in_=grow.rearrange("p (b k) -> p b k", k=NSEG),
                             axis=AX.X)
        logfin = const.tile([1, Bl], f32)
        nc.scalar.activation(logfin[:], fin_sb[:], AF.Ln)
        logm_v = logm.rearrange("p (b k) -> p k b", k=NSEG)
        logn_v = logn.rearrange("p (b k) -> p k b", k=NSEG)
        lz = const.tile([1, Bl], f32)
        nc.vector.tensor_add(out=lz[:], in0=growb[:], in1=logfin[:])
        nc.vector.tensor_tensor(out=lz[:], in0=lz[:],
                                in1=logm_v[:, NSEG - 1, :], op=OP.subtract)
        nc.vector.tensor_add(out=lz[:], in0=lz[:], in1=logn_v[:, 0, :])
        lzs = const.tile([1, 1], f32)
        nc.vector.reduce_sum(out=lzs[:], in_=lz[:], axis=AX.X)
        nps = pstat.tile([1, 512], f32, tag="st")
        nc.tensor.matmul(out=nps[:, :1], lhsT=ones_colf[:], rhs=numcol[:],
                         start=True, stop=True)
        res = const.tile([1, 1], f32)
        nc.vector.tensor_tensor(out=res[:], in0=nps[:, :1], in1=lzs[:],
                                op=OP.subtract)
        nc.sync.dma_start(out=outv[:, :], in_=res[:])

    nc.compile()
    return nc


def _get_nc():
    global _NC
    if _NC is None:
        _NC = _build()
    return _NC


def make_in_maps(inputs):
    em = np.asarray(inputs["emissions"], dtype=np.float32)
    tags = np.asarray(inputs["tags"]).astype(np.int32)
    st = np.asarray(inputs["start_transitions"], dtype=np.float32)
    en = np.asarray(inputs["end_transitions"], dtype=np.float32)
    tr = np.ascontiguousarray(np.asarray(inputs["transitions"],
                                         dtype=np.float32))
    tssev = np.concatenate(
        [tr.ravel(), st, en, np.zeros(1, np.float32)]).astype(
        np.float32).reshape(TSSE_N, 1)
    s_i = np.arange(S)[:, None]
    b_i = np.arange(Bl)[None, :]
    in_maps = []
    for c in range(NCORES):
        em_c = em[:, c * Bl:(c + 1) * Bl, :]
        # [s=(k,j), b, t] -> [t, j, k, b]; col = j*SLAB + k*Bl + b
        emr = np.ascontiguousarray(
            em_c.reshape(NSEG, SEGL, Bl, T).transpose(3, 1, 0, 2)
        ).reshape(T, NCOL)
        emT_b = emr.astype(ml_dtypes.bfloat16)
        tg = tags[:, c * Bl:(c + 1) * Bl]
        emi = ((s_i * Bl + b_i) * T + tg).astype(np.int32).reshape(128, 128)
        tse = np.full(128 * 129, TSSE_PAD, np.int32)
        tse[:511 * Bl] = (tg[:-1] * T + tg[1:]).astype(np.int32).ravel()
        tse[511 * Bl:511 * Bl + Bl] = T * T + tg[0]
        tse[511 * Bl + Bl:511 * Bl + 2 * Bl] = T * T + T + tg[-1]
        in_maps.append({
            "emT": emT_b,
            "emg": np.ascontiguousarray(em_c).reshape(S * Bl, T),
            "transm": tr,
            "startv": st.reshape(T, 1),
            "endv": en.reshape(T, 1),
            "emtidx": emi,
            "tssev": tssev,
            "tsseidx": tse.reshape(128, 129),
        })
    return in_maps


def _numpy_fallback(inputs):
    """Exact float64 port of the reference (handles arbitrary masks)."""
    em = np.asarray(inputs["emissions"], dtype=np.float64)
    tags = np.asarray(inputs["tags"]).astype(np.int64)
    mask = np.asarray(inputs["mask"]).astype(bool)
    st = np.asarray(inputs["start_transitions"], dtype=np.float64)
    en = np.asarray(inputs["end_transitions"], dtype=np.float64)
    tr = np.asarray(inputs["transitions"], dtype=np.float64)
    Sl, Bn = tags.shape
    mask_f = mask.astype(np.float64)
    emit = np.take_along_axis(em, tags[:, :, None], axis=2)[:, :, 0]
    trsc = tr[tags[:-1], tags[1:]]
    score = st[tags[0]] + emit[0]
    score = score + ((trsc + emit[1:]) * mask_f[1:]).sum(0)
    seq_ends = mask.astype(np.int64).sum(0) - 1
    score = score + en[tags[seq_ends, np.arange(Bn)]]
    alpha = st[None, :] + em[0]
    for s in range(1, Sl):
        nxt = alpha[:, :, None] + tr[None] + em[s][:, None, :]
        mx = nxt.max(axis=1)
        nxt = mx + np.log(np.exp(nxt - mx[:, None, :]).sum(axis=1))
        alpha = np.where(mask[s][:, None], nxt, alpha)
    z = alpha + en[None, :]
    mz = z.max(axis=1)
    logZ = mz + np.log(np.exp(z - mz[:, None]).sum(axis=1))
    return np.asarray((score - logZ).sum(), dtype=np.float32)


def run_device(inputs, trace=False, trace_kwargs=None):
    from concourse.bass_utils import run_bass_kernel_spmd
    nc = _get_nc()
    in_maps = make_in_maps(inputs)
    br = run_bass_kernel_spmd(nc, in_maps, list(range(NCORES)),
                              trace=trace, **(trace_kwargs or {}))
    total = np.float32(
        sum(float(br.results[i]["out"][0, 0]) for i in range(NCORES)))
    return np.asarray(total, dtype=np.float32), br


def kernel(**inputs):
    mask = np.asarray(inputs["mask"])
    if not bool(mask.all()):
        return _numpy_fallback(inputs)
    val, _ = run_device(inputs, trace=False)
    return val
